# revision 2
# baseline (speedup 1.0000x reference)
"""Bass/Tile kernel for a 3-layer bidirectional LSTM classifier on 8 TRN2 cores.

Problem shapes (hardcoded): x [256, 512, 16], H=256, 3 BiLSTM layers, fc -> [256].

Strategy: data-parallel over batch (B=32 per core, no collectives). Per core,
the fwd and rev recurrences of each layer run as two independent interleaved
streams. All state is kept transposed (hT [H, B], gates [4H, B]) so the
recurrence matmul keeps the recurrent weights as the PE-stationary operand and
no transposes are ever needed. Gate rows are pre-permuted host-side to
[i, f, o, g] chunk order so a single sigmoid covers chunks 0..5 and a single
tanh covers chunks 6..7.

Input projections (the large, parallel-over-T matmuls) are precomputed per
layer at N=512 in float32r (full PE rate) and staged in DRAM as bf16; the
sequential recurrence then only does the 16 small [128,128]x[128,32] fp32
matmuls + elementwise per step.
"""

import os
from contextlib import ExitStack

import numpy as np

import concourse.bass as bass
import concourse.mybir as mybir
import concourse.tile as tile
from concourse import bacc, bass_utils
from concourse.bass import ds

f32 = mybir.dt.float32
f32r = mybir.dt.float32r
bf16 = mybir.dt.bfloat16
AF = mybir.ActivationFunctionType

H = 256
G = 1024  # 4H
NCORES = 8
BFULL = 256
TFULL = 512
I0 = 16

# gate chunk order i,i,f,f,o,o,g,g (PyTorch order in weights is i,f,g,o)
_PERM = np.concatenate(
    [np.arange(0, 512), np.arange(768, 1024), np.arange(512, 768)]
)


def _prep_wih(w):
    """[1024, Din] -> stationary layout [min(Din,128), nk*1024], chunk (k, m)
    at cols k*1024 + m*128; lhsT[kk, m*128+mm] = w_perm[m*128+mm, k*128+kk]."""
    wr = np.asarray(w, np.float32)[_PERM]
    din = wr.shape[1]
    if din <= 128:
        return np.ascontiguousarray(wr.T)
    nk = din // 128
    out = np.empty((128, nk * 1024), np.float32)
    for k in range(nk):
        out[:, k * 1024 : (k + 1) * 1024] = wr[:, k * 128 : (k + 1) * 128].T
    return out


def _prep_b(b):
    return np.ascontiguousarray(np.asarray(b, np.float32)[_PERM].reshape(8, 128).T)


def build(nc, T=TFULL, B=32):
    """Emit the full per-core program into nc (a Bacc)."""
    TB = T * B
    U = 16  # rec unroll / proj block (timesteps)
    NBLK = T // U
    UB = U * B

    xT0 = nc.dram_tensor("xT0", [I0, TB], f32, kind="ExternalInput").ap()
    win = {}
    for l in range(3):
        kp, kch = (I0, 1) if l == 0 else (128, 4)
        for d, dn in enumerate("fr"):
            win[(l, d, "wih")] = nc.dram_tensor(
                f"wih{l}{dn}", [kp, kch * 1024], f32, kind="ExternalInput"
            ).ap()
            win[(l, d, "whh")] = nc.dram_tensor(
                f"whh{l}{dn}", [128, 2048], f32, kind="ExternalInput"
            ).ap()
            win[(l, d, "b")] = nc.dram_tensor(
                f"b{l}{dn}", [128, 8], f32, kind="ExternalInput"
            ).ap()
    out_h2f = nc.dram_tensor("h2f", [128, 2 * B], f32, kind="ExternalOutput").ap()
    out_h2r = nc.dram_tensor("h2r", [128, 2 * B], f32, kind="ExternalOutput").ap()

    with tile.TileContext(nc) as tc, ExitStack() as ctx:
        dram = ctx.enter_context(tc.tile_pool(name="dram", bufs=1, space="DRAM"))
        wpool = ctx.enter_context(tc.tile_pool(name="wts", bufs=1))
        mvpool = ctx.enter_context(tc.tile_pool(name="mv", bufs=3))
        pj_psum = ctx.enter_context(tc.tile_pool(name="pjps", bufs=2, space="PSUM"))
        xpspool = ctx.enter_context(tc.tile_pool(name="xps", bufs=3))
        xppool = ctx.enter_context(tc.tile_pool(name="xpt", bufs=4))
        rps = [
            ctx.enter_context(tc.tile_pool(name=f"rps{d}", bufs=2, space="PSUM"))
            for d in range(2)
        ]
        gpool = ctx.enter_context(tc.tile_pool(name="g", bufs=3))
        state = ctx.enter_context(tc.tile_pool(name="st", bufs=1))
        tmp = ctx.enter_context(tc.tile_pool(name="tmp", bufs=3))

        # DRAM scratch
        xp = {}
        for l in range(3):
            for d in range(2):
                nt = U if (l == 2 and d == 1) else T
                xp[(l, d)] = dram.tile([128, nt, 8 * B], bf16, tag=f"xp{l}{d}", name=f"xp{l}{d}")
        xin = {
            1: dram.tile([4, 128, TB], f32, tag="xin1", name="xin1"),
            2: dram.tile([4, 128, TB], f32, tag="xin2", name="xin2"),
        }

        def load_weights(l):
            kp, kch = (I0, 1) if l == 0 else (128, 4)
            wt = {}
            for d in range(2):
                wih_t = wpool.tile([kp, kch * 1024], f32r, tag=f"wih{d}")
                nc.sync.dma_start(wih_t[:], win[(l, d, "wih")][:].bitcast(f32r))
                whh_t = wpool.tile([128, 2048], f32, tag=f"whh{d}")
                nc.sync.dma_start(whh_t[:], win[(l, d, "whh")][:])
                b_t = wpool.tile([128, 8], f32, tag=f"b{d}")
                nc.sync.dma_start(b_t[:], win[(l, d, "b")][:])
                wt[d] = (wih_t, whh_t, b_t)
            return wt

        def emit_proj_block(l, wt, jb, dirs):
            """One real-time block jb (16 timesteps, UB cols) of the input
            projection for layer l, for the given target dirs."""
            kp, kch = (I0, 1) if l == 0 else (128, 4)
            mvs = []
            for k in range(kch):
                mv = mvpool.tile([kp, UB], f32r, tag=f"mv{k}")
                if l == 0:
                    nc.sync.dma_start(mv[:], xT0[:, ds(jb * UB, UB)].bitcast(f32r))
                else:
                    col = ds(jb * UB, UB) if k < 2 else ds(
                        (NBLK - 1) * UB - jb * UB, UB
                    )
                    nc.sync.dma_start(mv[:], xin[l][k, :, col].bitcast(f32r))
                mvs.append(mv)
            for d in dirs:
                wih_t, _, b_t = wt[d]
                for m in range(8):
                    ps = pj_psum.tile([128, UB], f32)
                    for k in range(kch):
                        straight = (d == 0) if (l == 0 or k < 2) else (d == 1)
                        rhs = (
                            mvs[k][:]
                            if straight
                            else mvs[k][:]
                            .rearrange("p (t b) -> p t b", b=B)[:, ::-1, :]
                        )
                        nc.tensor.matmul(
                            ps[:],
                            wih_t[:, (k * 8 + m) * 128 : (k * 8 + m + 1) * 128],
                            rhs,
                            start=(k == 0),
                            stop=(k == kch - 1),
                        )
                    xps = xpspool.tile([128, UB], bf16)
                    nc.scalar.activation(
                        xps[:], ps[:], AF.Identity, bias=b_t[:, m : m + 1]
                    )
                    dst_row = ds(jb * U, U) if d == 0 else ds(
                        (NBLK - 1) * U - jb * U, U
                    )
                    dst = xp[(l, d)][:, dst_row, m * B : (m + 1) * B]
                    src = xps[:].rearrange("p (t b) -> p t b", b=B)
                    nc.sync.dma_start(dst, src)

        def proj_layer(l, wt, dirs=(0, 1)):
            with tc.For_i(0, NBLK, 1) as jb:
                emit_proj_block(l, wt, jb, dirs)

        def cell_step(l, d, wt, jexpr, colexpr, h, c, store):
            _, whh_t, _ = wt[d]
            xpt = xppool.tile([128, 8 * B], bf16, tag=f"xp{d}")
            nc.sync.dma_start(xpt[:], xp[(l, d)][:, ds(jexpr, 1), :])
            ps = rps[d].tile([128, 8 * B], f32)
            for m in range(8):
                for k in range(2):
                    nc.tensor.matmul(
                        ps[:, m * B : (m + 1) * B],
                        whh_t[:, (k * 8 + m) * 128 : (k * 8 + m + 1) * 128],
                        h[:, k * B : (k + 1) * B],
                        start=(k == 0),
                        stop=(k == 1),
                    )
            g = gpool.tile([128, 8 * B], f32, tag=f"g{d}")
            nc.vector.tensor_add(g[:], ps[:], xpt[:])
            sg = gpool.tile([128, 6 * B], f32, tag=f"sg{d}")
            nc.scalar.activation(sg[:], g[:, 0 : 6 * B], AF.Sigmoid)
            tg = gpool.tile([128, 2 * B], f32, tag=f"tg{d}")
            nc.scalar.activation(tg[:], g[:, 6 * B : 8 * B], AF.Tanh)
            ta = tmp.tile([128, 2 * B], f32, tag=f"ta{d}")
            nc.gpsimd.tensor_mul(ta[:], sg[:, 2 * B : 4 * B], c[:])  # f*c
            tb = tmp.tile([128, 2 * B], f32, tag=f"tb{d}")
            nc.vector.tensor_mul(tb[:], sg[:, 0 : 2 * B], tg[:])  # i*g
            nc.vector.tensor_add(c[:], ta[:], tb[:])
            tcb = tmp.tile([128, 2 * B], f32, tag=f"tc{d}")
            nc.scalar.activation(tcb[:], c[:], AF.Tanh)
            nc.gpsimd.tensor_mul(h[:], sg[:, 4 * B : 6 * B], tcb[:])  # o*tanh(c)
            if store:
                for jh in range(2):
                    nc.sync.dma_start(
                        xin[l + 1][2 * d + jh, :, ds(colexpr, B)],
                        h[:, jh * B : (jh + 1) * B],
                    )

        def rec_layer(l, wt, dirs=(0, 1), store=True):
            hs, cs = {}, {}
            for d in dirs:
                hh = state.tile([128, 2 * B], f32, tag=f"h{d}")
                cc = state.tile([128, 2 * B], f32, tag=f"c{d}")
                nc.gpsimd.memset(hh[:], 0.0)
                nc.gpsimd.memset(cc[:], 0.0)
                hs[d], cs[d] = hh, cc
            with tc.For_i(
                0, T, U, hint_engines=(mybir.EngineType.PE,)
            ) as j0:
                jcol = j0 * B
                for s in range(U):
                    for d in dirs:
                        cell_step(
                            l, d, wt, j0 + s, jcol + s * B, hs[d], cs[d], store
                        )
            return hs, cs

        # ---- layer 0 ----
        wt = load_weights(0)
        proj_layer(0, wt)
        rec_layer(0, wt)
        # ---- layer 1 ----
        wt = load_weights(1)
        proj_layer(1, wt)
        rec_layer(1, wt)
        # ---- layer 2 ----
        wt = load_weights(2)
        proj_layer(2, wt, dirs=(0,))
        emit_proj_block(2, wt, NBLK - 1, dirs=(1,))
        hs, _ = rec_layer(2, wt, dirs=(0,), store=False)
        nc.sync.dma_start(out_h2f[:], hs[0][:])
        # layer-2 reverse: only its first step (t = T-1) feeds the output
        hr = state.tile([128, 2 * B], f32, tag="h1")
        cr = state.tile([128, 2 * B], f32, tag="c1")
        nc.gpsimd.memset(hr[:], 0.0)
        nc.gpsimd.memset(cr[:], 0.0)
        cell_step(2, 1, wt, 0, 0, hr, cr, False)
        nc.sync.dma_start(out_h2r[:], hr[:])


def _make_in_maps(inputs, T=TFULL, B=32, ncores=NCORES):
    x = np.ascontiguousarray(np.asarray(inputs["x"], np.float32))
    shared = {}
    for l in range(3):
        for d, dn in enumerate("fr"):
            shared[f"wih{l}{dn}"] = _prep_wih(inputs[f"wih{l}{dn}"])
            shared[f"whh{l}{dn}"] = _prep_wih(inputs[f"whh{l}{dn}"])
            shared[f"b{l}{dn}"] = _prep_b(inputs[f"b{l}{dn}"])
    in_maps = []
    for ci in range(ncores):
        xs = x[ci * B : (ci + 1) * B, :T]  # [B, T, 16]
        xt = np.ascontiguousarray(xs.transpose(2, 1, 0).reshape(I0, T * B))
        m = dict(shared)
        m["xT0"] = xt
        in_maps.append(m)
    return in_maps


def _assemble(results, inputs, B=32):
    fcw = np.asarray(inputs["fcw"], np.float32)[0]
    fcb = float(np.asarray(inputs["fcb"], np.float32)[0])
    out = np.empty(len(results) * B, np.float32)
    for ci, r in enumerate(results):
        h2f = np.concatenate([r["h2f"][:, :B], r["h2f"][:, B:]], axis=0)
        h2r = np.concatenate([r["h2r"][:, :B], r["h2r"][:, B:]], axis=0)
        out[ci * B : (ci + 1) * B] = fcw[:256] @ h2f + fcw[256:] @ h2r + fcb
    return out


def kernel(**inputs):
    nc = bacc.Bacc(
        "TRN2", target_bir_lowering=False, debug=False, num_devices=NCORES
    )
    build(nc)
    nc.compile()
    in_maps = _make_in_maps(inputs)
    trace = os.environ.get("KERNEL_TRACE", "0") == "1"
    res = bass_utils.run_bass_kernel_spmd(
        nc,
        in_maps,
        core_ids=list(range(NCORES)),
        trace=trace,
        tmpdir=os.environ.get("KERNEL_TRACE_DIR") if trace else None,
    )
    if trace and res.exec_time_ns is not None:
        print(f"HW exec time: {res.exec_time_ns} ns")
    return _assemble(res.results, inputs)



# revision 4
# speedup vs baseline: 3.1956x; 3.1956x over previous
"""Bass/Tile kernel for a 3-layer bidirectional LSTM classifier on 8 TRN2 cores.

Problem shapes (hardcoded): x [256, 512, 16], H=256, 3 BiLSTM layers, fc -> [256].

Strategy: data-parallel over batch (B=32 per core, no collectives). Per core,
the fwd and rev recurrences of each layer run as two independent interleaved
streams. All state is kept transposed (hT [H, B], gates [4H, B]) so the
recurrence matmul keeps the recurrent weights as the PE-stationary operand and
no transposes are ever needed. Gate rows are pre-permuted host-side to
[i, f, o, g] chunk order so a single sigmoid covers chunks 0..5 and a single
tanh covers chunks 6..7.

v2: everything on the PE path is bf16 (recurrent weights, hidden state,
input projections) so LDWEIGHTS runs with FWL and matmuls at 1 cycle/row —
the baseline's fp32 LDWEIGHTS+MATMUL pairs at ~512 ns dominated the whole
kernel. xp is loaded in 16-step slabs and h is staged in SBUF and stored
per-block to cut the per-step DMA/semaphore traffic. Cell state c stays
fp32; the final h that feeds the classifier is computed in fp32.
"""

import os
from contextlib import ExitStack

import numpy as np
import ml_dtypes

import concourse.bass as bass
import concourse.mybir as mybir
import concourse.tile as tile
from concourse import bacc, bass_utils
from concourse.bass import ds

f32 = mybir.dt.float32
bf16 = mybir.dt.bfloat16
AF = mybir.ActivationFunctionType
np_bf16 = ml_dtypes.bfloat16

H = 256
G = 1024  # 4H
NCORES = 8
BFULL = 256
TFULL = 512
I0 = 16

# gate chunk order i,i,f,f,o,o,g,g (PyTorch order in weights is i,f,g,o)
_PERM = np.concatenate(
    [np.arange(0, 512), np.arange(768, 1024), np.arange(512, 768)]
)


def _prep_wih(w):
    """[1024, Din] -> stationary layout [min(Din,128), nk*1024], chunk (k, m)
    at cols k*1024 + m*128; lhsT[kk, m*128+mm] = w_perm[m*128+mm, k*128+kk]."""
    wr = np.asarray(w, np.float32)[_PERM]
    din = wr.shape[1]
    if din <= 128:
        return np.ascontiguousarray(wr.T.astype(np_bf16))
    nk = din // 128
    out = np.empty((128, nk * 1024), np_bf16)
    for k in range(nk):
        out[:, k * 1024 : (k + 1) * 1024] = wr[:, k * 128 : (k + 1) * 128].T
    return out


def _prep_b(b):
    return np.ascontiguousarray(np.asarray(b, np.float32)[_PERM].reshape(8, 128).T)


def build(nc, T=TFULL, B=32):
    """Emit the full per-core program into nc (a Bacc)."""
    TB = T * B
    U = 16  # rec unroll / proj block (timesteps)
    NBLK = T // U
    UB = U * B

    xT0 = nc.dram_tensor("xT0", [I0, TB], bf16, kind="ExternalInput").ap()
    win = {}
    for l in range(3):
        kp, kch = (I0, 1) if l == 0 else (128, 4)
        for d, dn in enumerate("fr"):
            win[(l, d, "wih")] = nc.dram_tensor(
                f"wih{l}{dn}", [kp, kch * 1024], bf16, kind="ExternalInput"
            ).ap()
            win[(l, d, "whh")] = nc.dram_tensor(
                f"whh{l}{dn}", [128, 2048], bf16, kind="ExternalInput"
            ).ap()
            win[(l, d, "b")] = nc.dram_tensor(
                f"b{l}{dn}", [128, 8], f32, kind="ExternalInput"
            ).ap()
    out_h2f = nc.dram_tensor("h2f", [128, 2 * B], f32, kind="ExternalOutput").ap()
    out_h2r = nc.dram_tensor("h2r", [128, 2 * B], f32, kind="ExternalOutput").ap()

    with tile.TileContext(nc) as tc, ExitStack() as ctx:
        dram = ctx.enter_context(tc.tile_pool(name="dram", bufs=1, space="DRAM"))
        wpool = ctx.enter_context(tc.tile_pool(name="wts", bufs=1))
        mvpool = ctx.enter_context(tc.tile_pool(name="mv", bufs=3))
        pj_psum = ctx.enter_context(tc.tile_pool(name="pjps", bufs=2, space="PSUM"))
        xpspool = ctx.enter_context(tc.tile_pool(name="xps", bufs=3))
        slabpool = ctx.enter_context(tc.tile_pool(name="slab", bufs=2))
        rps = [
            ctx.enter_context(tc.tile_pool(name=f"rps{d}", bufs=2, space="PSUM"))
            for d in range(2)
        ]
        gpool = ctx.enter_context(tc.tile_pool(name="g", bufs=3))
        state = ctx.enter_context(tc.tile_pool(name="st", bufs=1))
        tmp = ctx.enter_context(tc.tile_pool(name="tmp", bufs=3))

        # DRAM scratch
        xp = {}
        for l in range(3):
            for d in range(2):
                nb = 1 if (l == 2 and d == 1) else NBLK
                xp[(l, d)] = dram.tile(
                    [128, nb, 8, UB], bf16, tag=f"xp{l}{d}", name=f"xp{l}{d}"
                )
        xin = {
            1: dram.tile([4, 128, TB], bf16, tag="xin1", name="xin1"),
            2: dram.tile([4, 128, TB], bf16, tag="xin2", name="xin2"),
        }

        def load_weights(l):
            kp, kch = (I0, 1) if l == 0 else (128, 4)
            wt = {}
            for d in range(2):
                wih_t = wpool.tile([kp, kch * 1024], bf16, tag=f"wih{d}")
                nc.sync.dma_start(wih_t[:], win[(l, d, "wih")][:])
                whh_t = wpool.tile([128, 2048], bf16, tag=f"whh{d}")
                nc.sync.dma_start(whh_t[:], win[(l, d, "whh")][:])
                b_t = wpool.tile([128, 8], f32, tag=f"b{d}")
                nc.sync.dma_start(b_t[:], win[(l, d, "b")][:])
                wt[d] = (wih_t, whh_t, b_t)
            return wt

        def emit_proj_block(l, wt, jb, dirs):
            """One real-time block jb (16 timesteps, UB cols) of the input
            projection for layer l, for the given target dirs."""
            kp, kch = (I0, 1) if l == 0 else (128, 4)
            mvs = []
            for k in range(kch):
                mv = mvpool.tile([kp, UB], bf16, tag=f"mv{k}")
                if l == 0:
                    nc.sync.dma_start(mv[:], xT0[:, ds(jb * UB, UB)])
                else:
                    col = ds(jb * UB, UB) if k < 2 else ds(
                        (NBLK - 1) * UB - jb * UB, UB
                    )
                    nc.sync.dma_start(mv[:], xin[l][k, :, col])
                mvs.append(mv)
            for d in dirs:
                wih_t, _, b_t = wt[d]
                for m in range(8):
                    ps = pj_psum.tile([128, UB], f32)
                    for k in range(kch):
                        straight = (d == 0) if (l == 0 or k < 2) else (d == 1)
                        rhs = (
                            mvs[k][:]
                            if straight
                            else mvs[k][:]
                            .rearrange("p (t b) -> p t b", b=B)[:, ::-1, :]
                        )
                        nc.tensor.matmul(
                            ps[:],
                            wih_t[:, (k * 8 + m) * 128 : (k * 8 + m + 1) * 128],
                            rhs,
                            start=(k == 0),
                            stop=(k == kch - 1),
                        )
                    xps = xpspool.tile([128, UB], bf16)
                    nc.scalar.activation(
                        xps[:], ps[:], AF.Identity, bias=b_t[:, m : m + 1]
                    )
                    jblk = jb if d == 0 else (NBLK - 1) - jb
                    if l == 2 and d == 1:
                        jblk = 0
                    nc.sync.dma_start(xp[(l, d)][:, ds(jblk, 1), m, :], xps[:])

        def proj_layer(l, wt, dirs=(0, 1)):
            with tc.For_i(0, NBLK, 1) as jb:
                emit_proj_block(l, wt, jb, dirs)

        def cell_step(d, wt, s, slab, hhist, cc, fin=None):
            """One recurrence step for direction d at in-block offset s.
            Reads h from hhist[:, :, s-1, :] (U-1 wraps to previous block),
            writes h to hhist[:, :, s, :]."""
            _, whh_t, _ = wt[d]
            sp = U - 1 if s == 0 else s - 1
            hprev = hhist[:, :, sp, :]
            ps = rps[d].tile([128, 8, B], f32)
            for m in range(8):
                for k in range(2):
                    nc.tensor.matmul(
                        ps[:, m, :],
                        whh_t[:, (k * 8 + m) * 128 : (k * 8 + m + 1) * 128],
                        hprev[:, k, :],
                        start=(k == 0),
                        stop=(k == 1),
                    )
            g = gpool.tile([128, 8, B], f32, tag=f"g{d}")
            nc.vector.tensor_add(g[:], ps[:], slab[:, :, ds(s * B, B)])
            gf = g[:].rearrange("p m b -> p (m b)")
            sg = gpool.tile([128, 6 * B], f32, tag=f"sg{d}")
            nc.scalar.activation(sg[:], gf[:, 0 : 6 * B], AF.Sigmoid)
            tg = gpool.tile([128, 2 * B], f32, tag=f"tg{d}")
            nc.scalar.activation(tg[:], gf[:, 6 * B : 8 * B], AF.Tanh)
            ta = tmp.tile([128, 2 * B], f32, tag=f"ta{d}")
            nc.gpsimd.tensor_mul(ta[:], sg[:, 2 * B : 4 * B], cc[:])  # f*c
            tb = tmp.tile([128, 2 * B], f32, tag=f"tb{d}")
            nc.vector.tensor_mul(tb[:], sg[:, 0 : 2 * B], tg[:])  # i*g
            nc.vector.tensor_add(cc[:], ta[:], tb[:])
            tcb = tmp.tile([128, 2 * B], f32, tag=f"tc{d}")
            nc.scalar.activation(tcb[:], cc[:], AF.Tanh)
            hv = hhist[:, :, s, :]
            og = sg[:, 4 * B : 6 * B].rearrange("p (k b) -> p k b", b=B)
            tc2 = tcb[:].rearrange("p (k b) -> p k b", b=B)
            nc.gpsimd.tensor_mul(hv, og, tc2)  # o*tanh(c) -> bf16 h
            if fin is not None:
                nc.vector.tensor_mul(fin[:], sg[:, 4 * B : 6 * B], tcb[:])

        def rec_block(l, dirs, wt, hh, cs, jb, store, fin=None):
            slabs = {}
            for d in dirs:
                slab = slabpool.tile([128, 8, UB], bf16, tag=f"slab{d}")
                nc.sync.dma_start(slab[:], xp[(l, d)][:, ds(jb, 1), :, :])
                slabs[d] = slab
            for s in range(U):
                for d in dirs:
                    f = fin if (fin is not None and s == U - 1 and d == 0) else None
                    cell_step(d, wt, s, slabs[d], hh[d], cs[d], fin=f)
            if store:
                for d in dirs:
                    for k in range(2):
                        nc.sync.dma_start(
                            xin[l + 1][2 * d + k, :, ds(jb * UB, UB)],
                            hh[d][:, k, :, :].rearrange("p u b -> p (u b)"),
                        )

        def rec_layer2(l, wt, dirs=(0, 1), store=True, fin=None):
            hh, cs = {}, {}
            for d in dirs:
                hhist = state.tile([128, 2, U, B], bf16, tag=f"h{d}")
                cc = state.tile([128, 2 * B], f32, tag=f"c{d}")
                nc.gpsimd.memset(hhist[:], 0.0)
                nc.gpsimd.memset(cc[:], 0.0)
                hh[d], cs[d] = hhist, cc
            nlast = 1 if fin is not None else 0
            with tc.For_i(
                0, NBLK - nlast, 1, hint_engines=(mybir.EngineType.PE,)
            ) as jb:
                rec_block(l, dirs, wt, hh, cs, jb, store)
            if nlast:
                rec_block(l, dirs, wt, hh, cs, NBLK - 1, store, fin=fin)
            return hh, cs

        # ---- layer 0 ----
        wt = load_weights(0)
        proj_layer(0, wt)
        rec_layer2(0, wt)
        # ---- layer 1 ----
        wt = load_weights(1)
        proj_layer(1, wt)
        rec_layer2(1, wt)
        # ---- layer 2 ----
        wt = load_weights(2)
        proj_layer(2, wt, dirs=(0,))
        emit_proj_block(2, wt, NBLK - 1, dirs=(1,))
        hfin = state.tile([128, 2 * B], f32, tag="hfin")
        rec_layer2(2, wt, dirs=(0,), store=False, fin=hfin)
        nc.sync.dma_start(out_h2f[:], hfin[:])
        # layer-2 reverse: only its first step (t = T-1) feeds the output.
        # h_prev = c_prev = 0, so gates = xp directly and c = i*g.
        xpt = tmp.tile([128, 8, B], bf16, tag="l2r_xp")
        nc.sync.dma_start(xpt[:], xp[(2, 1)][:, 0, :, ds(0, B)])
        gf = xpt[:].rearrange("p m b -> p (m b)")
        sg = gpool.tile([128, 6 * B], f32, tag="l2r_sg")
        nc.scalar.activation(sg[:], gf[:, 0 : 6 * B], AF.Sigmoid)
        tg = gpool.tile([128, 2 * B], f32, tag="l2r_tg")
        nc.scalar.activation(tg[:], gf[:, 6 * B : 8 * B], AF.Tanh)
        cr = state.tile([128, 2 * B], f32, tag="l2r_c")
        nc.vector.tensor_mul(cr[:], sg[:, 0 : 2 * B], tg[:])  # c = i*g
        tcb = tmp.tile([128, 2 * B], f32, tag="l2r_tc")
        nc.scalar.activation(tcb[:], cr[:], AF.Tanh)
        hr = state.tile([128, 2 * B], f32, tag="l2r_h")
        nc.vector.tensor_mul(hr[:], sg[:, 4 * B : 6 * B], tcb[:])
        nc.sync.dma_start(out_h2r[:], hr[:])


def _make_in_maps(inputs, T=TFULL, B=32, ncores=NCORES):
    x = np.asarray(inputs["x"], np.float32)
    shared = {}
    for l in range(3):
        for d, dn in enumerate("fr"):
            shared[f"wih{l}{dn}"] = _prep_wih(inputs[f"wih{l}{dn}"])
            shared[f"whh{l}{dn}"] = _prep_wih(inputs[f"whh{l}{dn}"])
            shared[f"b{l}{dn}"] = _prep_b(inputs[f"b{l}{dn}"])
    in_maps = []
    for ci in range(ncores):
        xs = x[ci * B : (ci + 1) * B, :T]  # [B, T, 16]
        xt = np.ascontiguousarray(
            xs.transpose(2, 1, 0).reshape(I0, T * B).astype(np_bf16)
        )
        m = dict(shared)
        m["xT0"] = xt
        in_maps.append(m)
    return in_maps


def _assemble(results, inputs, B=32):
    fcw = np.asarray(inputs["fcw"], np.float32)[0]
    fcb = float(np.asarray(inputs["fcb"], np.float32)[0])
    out = np.empty(len(results) * B, np.float32)
    for ci, r in enumerate(results):
        h2f = np.concatenate([r["h2f"][:, :B], r["h2f"][:, B:]], axis=0)
        h2r = np.concatenate([r["h2r"][:, :B], r["h2r"][:, B:]], axis=0)
        out[ci * B : (ci + 1) * B] = fcw[:256] @ h2f + fcw[256:] @ h2r + fcb
    return out


def kernel(**inputs):
    nc = bacc.Bacc(
        "TRN2", target_bir_lowering=False, debug=False, num_devices=NCORES
    )
    build(nc)
    nc.compile()
    in_maps = _make_in_maps(inputs)
    trace = os.environ.get("KERNEL_TRACE", "0") == "1"
    res = bass_utils.run_bass_kernel_spmd(
        nc,
        in_maps,
        core_ids=list(range(NCORES)),
        trace=trace,
        tmpdir=os.environ.get("KERNEL_TRACE_DIR") if trace else None,
    )
    if trace and res.exec_time_ns is not None:
        print(f"HW exec time: {res.exec_time_ns} ns")
    return _assemble(res.results, inputs)
